# revision 31
# baseline (speedup 1.0000x reference)
"""Trainium2 Bass kernel for nn_BinnedLoss (tent-weighted 128-bin chi2 loss).

O(N) factorized histogram: bin k = 8*q + r (q in [0,16), r in [0,8)).
Per 8-column block, DVE builds a q-one-hot [128,16*8] (stationary) and a
value matrix [128,16*8] = [roh*va | roh*vb] (moving); one PE matmul per
block accumulates all (q, r, stream) sums into a [128,128] PSUM tile whose
8 diagonal 16x16 sub-blocks hold the histogram. Tent weights: sample with
u=(x-mn)/step contributes w*(1-frac) to bin kf and w*frac to bin kf+1
(continuity makes exact-floor ties irrelevant). Gather-matmuls extract the
diagonal blocks, one AllReduce combines cores, chi2 tail mirrors reference.

kernel(**inputs) -> np.float32 scalar (shape ()).
"""
import os
import sys

sys.path.insert(0, "/opt/trn_rl_repo")
import numpy as np

N = 16777216
NCORES = 8
BINS = 128
P = 128
NSH = N // NCORES            # samples per core
FTOT = NSH // P              # 16384 free columns per core per array
FC = 1024                    # columns per chunk
NBLK = FC // 8               # matmul blocks per chunk
NCH = FTOT // FC             # chunks per array
MAGIC2 = 12582912.0          # 1.5*2^23: round-to-int magic for [0,128) range
QBIAS = -1572864.0           # -(12582912 >> 3)


def _patches(mybir, tile):
    from concourse.vector_clock import ScopedClock

    def _patched(self, tick_clock, wait_clock):
        drain_inst = self.nc.sync.drain()
        wait_clock.add_sem_waits(
            drain_inst.ins, ScopedClock({None: tick_clock.global_clock})
        )
        si = drain_inst.ins.sync_info
        if si is not None and si.on_wait and len(si.on_wait) > 1:
            waits = list(si.on_wait)
            drain_inst.ins.sync_info = mybir.SyncInfo(
                on_wait=[waits[0]], on_update=list(si.on_update)
            )
            for w in waits[1:]:
                nop = self.nc.sync.nop()
                nop.ins.sync_info = mybir.SyncInfo(on_wait=[w], on_update=[])
        self.nc.all_engine_barrier()
        assert self.sems is not None
        popped = self.nc._tile_sem_poison_stack.pop()
        assert popped is self._sem_poison
        self.nc.clear_and_free_semaphores(list(self.sems.allocated().values()))
        self.nc.all_engine_barrier()

    tile.TileContext._drain_and_barrier = _patched


def _split_sync_waits(nc, mybir, strip_same_engine=True):
    """Drop same-engine waits; hoist extra sem-waits onto same-engine NOPs
    (walrus allows <=1 sem-wait per instruction)."""
    eng_sem = {}
    counter = [0]
    for f in nc.m.functions:
        for bb in f.blocks:
            out = []
            dirty = False
            for inst in bb.instructions:
                si = inst.sync_info
                pref = eng_sem.get(inst.engine) if strip_same_engine else None
                if si is not None and si.on_wait and pref is not None:
                    kept = [
                        w for w in si.on_wait
                        if not (w.ant_name or "").startswith(pref + "_")
                    ]
                    if len(kept) != len(si.on_wait):
                        inst.sync_info = mybir.SyncInfo(
                            on_wait=kept, on_update=list(si.on_update))
                        si = inst.sync_info
                        dirty = True
                if si is not None and si.on_wait and len(si.on_wait) > 1:
                    waits = list(si.on_wait)
                    for w in waits[:-1]:
                        counter[0] += 1
                        nop = mybir.InstNoOp(
                            name=f"WSPLIT-{counter[0]}", ins=[], outs=[]
                        )
                        nop.engine = inst.engine
                        nop.sync_info = mybir.SyncInfo(on_wait=[w], on_update=[])
                        nc.register_instruction(nop, overwrite=True)
                        out.append(nop)
                    inst.sync_info = mybir.SyncInfo(
                        on_wait=[waits[-1]], on_update=list(si.on_update)
                    )
                    dirty = True
                out.append(inst)
            if dirty:
                bb.instructions = out


def build(repeat=1, strip_waits=True, CHUNK_A=4096, NPS=5, NPE=3):
    import concourse.bass as bass
    import concourse.mybir as mybir
    from concourse import tile

    _patches(mybir, tile)
    DT = mybir.dt
    AL = mybir.AluOpType
    ACT = mybir.ActivationFunctionType
    F32 = DT.float32
    BF16 = DT.bfloat16
    I32 = DT.int32
    core_ids = list(range(NCORES))

    nc = bass.Bass()
    sim_ext = nc.declare_dram_parameter("sim", [P, FTOT], F32, isOutput=False)
    exp_ext = nc.declare_dram_parameter("exp", [P, FTOT], F32, isOutput=False)
    w_ext = nc.declare_dram_parameter("w", [P, FTOT], F32, isOutput=False)
    out_ext = nc.declare_dram_parameter("out", [1, 1], F32, isOutput=True)

    with tile.TileContext(nc) as tc:
        with (
            tc.tile_pool(name="const", bufs=1) as cpool,
            tc.tile_pool(name="dram", bufs=1, space="DRAM") as dram,
            tc.tile_pool(name="psum", bufs=1, space="PSUM") as psum,
        ):
            cc_a_in = dram.tile([1, 2], F32, name="cc_a_in")
            cc_a_out = dram.tile([1, 16], F32, name="cc_a_out")
            cc_h_in = [dram.tile([1, 256], F32, name=f"cc_h_in{a}")
                       for a in range(2)]
            cc_h_out = [dram.tile([1, 2048], F32, name=f"cc_h_out{a}")
                        for a in range(2)]

            ones1 = cpool.tile([1, P], F32, name="ones1")
            nc.vector.memset(ones1[:], 1.0)
            # gather masks pdc[p, dc, jq] = (p == jq*8+dc)
            pidx = cpool.tile([P, 1], I32, name="pidx")
            nc.gpsimd.iota(pidx[:], [[1, 1]], channel_multiplier=1)
            pfl = cpool.tile([P, 1], F32, name="pfl")
            nc.vector.tensor_copy(pfl[:], pidx[:])
            pdc = cpool.tile([P, 8, 16], F32, name="pdc")
            for dc in range(8):
                for jq in range(16):
                    nc.vector.tensor_scalar(
                        pdc[:, dc, jq:jq + 1], pfl[:], float(jq * 8 + dc),
                        None, AL.is_equal)

            # scalars: sc = [inv, bias0]; dsc = delta
            sc = cpool.tile([1, 2], F32, name="sc")
            dsc = cpool.tile([1, 1], F32, name="dsc")
            bc = cpool.tile([P, 2], F32, name="bc")
            bcps = psum.tile([P, 2], F32, name="bcps")
            psA = psum.tile([P, P], F32, name="psA")
            psB = psum.tile([P, P], F32, name="psB")
            g0 = psum.tile([16, P], F32, name="g0")
            g1 = psum.tile([16, P], F32, name="g1")
            g2 = psum.tile([1, 256], F32, name="g2")

            cm2 = cpool.tile([P, 1], F32, name="cm2")
            nc.vector.memset(cm2[:], MAGIC2)
            cm2n = cpool.tile([P, 1], F32, name="cm2n")
            nc.vector.memset(cm2n[:], -MAGIC2)
            cq1 = cpool.tile([P, 1], F32, name="cq1")
            nc.vector.memset(cq1[:], -0.4375)
            cq2 = cpool.tile([P, 1], F32, name="cq2")
            nc.vector.memset(cq2[:], 0.125)
            gh = cpool.tile([1, 32, 16], F32, name="gh")
            onescol = cpool.tile([P, 1], F32, name="onescol")
            nc.vector.memset(onescol[:], 1.0)
            accs = [cpool.tile([16, 16], F32, name=f"acc{a}") for a in range(2)]

            for rep in range(repeat):
                # ---------------- Phase A: global min/max ----------------
                with tc.tile_pool(name=f"pa{rep}", bufs=2) as pa:
                    CW = CHUNK_A
                    rmin = pa.tile([P, 1], F32, name="rmin", bufs=1)
                    rmax = pa.tile([P, 1], F32, name="rmax", bufs=1)
                    nc.vector.memset(rmin[:], 1.0e30)
                    nc.vector.memset(rmax[:], -1.0e30)
                    for cv in range(0, FTOT, CW):
                        chs = pa.tile([P, CW], F32, name="chs", tag="chs")
                        che = pa.tile([P, CW], F32, name="che", tag="che")
                        tmin = pa.tile([P, 1], F32, name="tmin", bufs=1)
                        tmax = pa.tile([P, 1], F32, name="tmax", bufs=1)
                        nc.sync.dma_start(chs[:], sim_ext[:, bass.ds(cv, CW)])
                        nc.sync.dma_start(che[:], exp_ext[:, bass.ds(cv, CW)])
                        for ch in (chs, che):
                            nc.vector.tensor_reduce(
                                tmin[:], ch[:], mybir.AxisListType.X, AL.min)
                            nc.vector.tensor_reduce(
                                tmax[:], ch[:], mybir.AxisListType.X, AL.max)
                            nc.vector.tensor_tensor(
                                rmin[:], rmin[:], tmin[:], AL.min)
                            nc.vector.tensor_tensor(
                                rmax[:], rmax[:], tmax[:], AL.max)
                    pm = pa.tile([1, 2 * P], F32, name="pm", bufs=1)
                    nc.gpsimd.dma_start(pm[0:1, 0:P], rmax[:, 0:1])
                    nc.gpsimd.dma_start(pm[0:1, P:2 * P], rmin[:, 0:1])
                    pk = pa.tile([1, 2], F32, name="pk", bufs=1)
                    nc.vector.tensor_reduce(
                        pk[0:1, 0:1], pm[0:1, 0:P], mybir.AxisListType.X, AL.max)
                    nc.vector.tensor_reduce(
                        pk[0:1, 1:2], pm[0:1, P:2 * P], mybir.AxisListType.X, AL.min)
                    nc.vector.tensor_scalar_mul(pk[0:1, 1:2], pk[0:1, 1:2], -1.0)
                    nc.gpsimd.dma_start(cc_a_in[:], pk[:])
                    nc.gpsimd.collective_compute(
                        "AllGather", AL.bypass, replica_groups=[core_ids],
                        ins=[cc_a_in.opt()], outs=[cc_a_out.opt()],
                    )
                    ga_all = pa.tile([1, 16], F32, name="ga_all", bufs=1)
                    nc.gpsimd.dma_start(ga_all[:], cc_a_out[:])
                    ga = pa.tile([1, 2], F32, name="ga", bufs=1)
                    nc.vector.tensor_reduce(
                        ga[0:1, 0:1], ga_all[0:1, bass.ds(0, 8, 2)],
                        mybir.AxisListType.X, AL.max)
                    nc.vector.tensor_reduce(
                        ga[0:1, 1:2], ga_all[0:1, bass.ds(1, 8, 2)],
                        mybir.AxisListType.X, AL.max)
                    # ga = [mx, -mn]; d = mx - mn
                    mnt = pa.tile([1, 1], F32, name="mnt", bufs=1)
                    d_t = pa.tile([1, 1], F32, name="d_t", bufs=1)
                    st_t = pa.tile([1, 1], F32, name="st_t", bufs=1)
                    nc.vector.tensor_scalar_mul(mnt[:], ga[0:1, 1:2], -1.0)
                    nc.vector.tensor_tensor(d_t[:], ga[0:1, 0:1], mnt[:], AL.subtract)
                    nc.vector.tensor_scalar_mul(
                        st_t[:], d_t[:], float(np.float32(1.0) / np.float32(127.0)))
                    nc.vector.reciprocal(sc[0:1, 0:1], st_t[:])      # inv
                    nc.vector.scalar_tensor_tensor(
                        sc[0:1, 1:2], mnt[:], -1.0, sc[0:1, 0:1],
                        AL.mult, AL.mult)                            # -mn*inv
                    nc.vector.tensor_scalar_add(sc[0:1, 1:2], sc[0:1, 1:2], -0.5)
                    nc.vector.tensor_scalar_mul(dsc[:], d_t[:], 0.0078125)
                    nc.tensor.matmul(bcps[:], ones1[:], sc[0:1, :],
                                     start=True, stop=True)
                    nc.vector.tensor_copy(bc[:], bcps[:])

                # ---------------- Phase B ----------------
                ARR = ((sim_ext, True, psA), (exp_ext, False, psB))
                with (
                    tc.tile_pool(name=f"st{rep}", bufs=2) as st,
                    tc.tile_pool(name=f"pw{rep}", bufs=2) as pw,
                    tc.tile_pool(name=f"oh{rep}", bufs=2) as oh,
                    tc.tile_pool(name=f"px{rep}", bufs=1) as px,
                ):
                    # two-stage software pipeline per array: stage1(ci) feeds
                    # the engines (DMA, Act scale/bias chain, casts); stage2
                    # (ci-1) consumes (DVE one-hots + a-stream, Pool b-stream
                    # mults, PE matmuls). The sim histogram's extraction and
                    # AllGather are emitted two chunks into the exp pipeline
                    # so the collective overlaps exp compute.
                    casts = {}

                    def stage1(ai, ci):
                        arr, weighted, ps = ARR[ai]
                        x = st.tile([P, FC], F32, name="x", tag="x")
                        nc.sync.dma_start(x[:], arr[:, bass.ds(ci * FC, FC)])
                        wt = None
                        if weighted:
                            wt = st.tile([P, FC], F32, name="wt", tag="wt")
                            nc.sync.dma_start(
                                wt[:], w_ext[:, bass.ds(ci * FC, FC)])
                        u = pw.tile([P, FC], F32, name="u", tag="u")
                        t = pw.tile([P, FC], F32, name="t", tag="t", bufs=1)
                        kf = pw.tile([P, FC], F32, name="kf", tag="kf", bufs=1)
                        fr = pw.tile([P, FC], F32, name="fr", tag="fr", bufs=1)
                        ki = pw.tile([P, FC], I32, name="ki", tag="ki", bufs=1)
                        qi_f = pw.tile([P, FC], F32, name="qi_f", tag="qi", bufs=1)
                        qi_f2 = pw.tile([P, FC], F32, name="qi_f2", tag="qi2", bufs=1)
                        ri = pw.tile([P, FC], I32, name="ri", tag="ri", bufs=1)
                        qb = pw.tile([P, NBLK, 8], BF16, name="qb", tag="qb")
                        rb = pw.tile([P, NBLK, 8], BF16, name="rb", tag="rb")
                        nc.scalar.activation(
                            u[:], x[:], ACT.Identity,
                            bias=bc[:, 1:2], scale=bc[:, 0:1])
                        nc.scalar.activation(t[:], u[:], ACT.Identity,
                                             bias=cm2[:, 0:1])
                        nc.scalar.activation(kf[:], t[:], ACT.Identity,
                                             bias=cm2n[:, 0:1])
                        nc.scalar.activation(ki[:], t[:], ACT.Copy)
                        nc.vector.tensor_scalar(
                            ri[:], ki[:], 7, None, AL.bitwise_and)
                        # q via Act-only: a1 = kf/8 - 0.4375 (exact),
                        # a2 = round(a1) + M2 (magic), qb = a2 - M2
                        nc.scalar.activation(qi_f[:], kf[:], ACT.Identity,
                                             bias=cq1[:, 0:1], scale=cq2[:, 0:1])
                        nc.scalar.activation(qi_f2[:], qi_f[:], ACT.Identity,
                                             bias=cm2[:, 0:1])
                        nc.scalar.activation(qb[:], qi_f2[:], ACT.Copy,
                                             bias=-MAGIC2)
                        nc.scalar.activation(rb[:], ri[:], ACT.Copy)
                        nc.gpsimd.tensor_tensor(fr[:], u[:], kf[:], AL.subtract)
                        fb = pw.tile([P, NBLK, 8], BF16, name="fb", tag="fb")
                        nc.scalar.activation(fb[:], fr[:], ACT.Copy, bias=0.5)
                        va = None
                        if weighted:
                            va = pw.tile([P, NBLK, 8], BF16, name="va", tag="va")
                            nc.scalar.activation(va[:], wt[:], ACT.Copy)
                        casts[(ai, ci)] = (qb, rb, fb, va)

                    def stage2(ai, ci):
                        arr, weighted, ps = ARR[ai]
                        qb, rb, fb, va = casts.pop((ai, ci))
                        QOH = oh.tile([P, NBLK, 16, 8], BF16, name="QOH", tag="QOH")
                        V = oh.tile([P, NBLK, 16, 8], BF16, name="V", tag="V")
                        for jq in range(16):
                            nc.vector.tensor_scalar(
                                QOH[:, :, jq, :], qb[:], float(jq),
                                None, AL.is_equal)
                        if weighted:
                            ROH = oh.tile([P, NBLK, 8], BF16, name="ROH",
                                          tag="ROH", bufs=1)
                            for jr in range(8):
                                nc.vector.tensor_scalar(
                                    ROH[:], rb[:], float(jr), None, AL.is_equal)
                                nc.vector.tensor_tensor(
                                    V[:, :, jr, :], ROH[:], va[:], AL.mult)
                        else:
                            for jr in range(8):
                                nc.vector.tensor_scalar(
                                    V[:, :, jr, :], rb[:], float(jr),
                                    None, AL.is_equal)
                        npool = NPS if weighted else NPE
                        for jr in range(8):
                            eng = nc.gpsimd if jr < npool else nc.vector
                            eng.tensor_tensor(
                                V[:, :, 8 + jr, :], V[:, :, jr, :],
                                fb[:], AL.mult)
                        for b in range(NBLK):
                            nc.tensor.matmul(
                                ps[:, :], QOH[:, b, :, :], V[:, b, :, :],
                                start=(ci == 0 and b == 0),
                                stop=(ci == NCH - 1 and b == NBLK - 1))

                    def extract(ai):
                        # diagonal blocks acc[jq,jv] = sum_dc ps[jq*8+dc, jv*8+dc],
                        # then AllGather + cross-core sum into gh rows
                        ps = ARR[ai][2]
                        Sf = px.tile([P, P], F32, name="Sf", tag="Sf")
                        nc.vector.tensor_copy(Sf[:], ps[:, :])
                        acc = accs[ai]
                        for dc in range(8):
                            g = (g0, g1)[dc % 2]
                            nc.tensor.matmul(g[:, :], pdc[:, dc, :], Sf[:, :],
                                             start=True, stop=True)
                            if dc == 0:
                                nc.vector.tensor_copy(acc[:], g[:, bass.ds(dc, 16, 8)])
                            else:
                                nc.vector.tensor_tensor(
                                    acc[:], acc[:], g[:, bass.ds(dc, 16, 8)], AL.add)
                        nc.gpsimd.dma_start(cc_h_in[ai][:], acc[:, :])
                        nc.gpsimd.collective_compute(
                            "AllGather", AL.bypass, replica_groups=[core_ids],
                            ins=[cc_h_in[ai].opt()], outs=[cc_h_out[ai].opt()],
                        )
                        gh_all = px.tile([8, 256], F32, name="gh_all",
                                         tag="gh_all")
                        nc.gpsimd.dma_start(gh_all[:], cc_h_out[ai][:])
                        nc.tensor.matmul(g2[:, :], onescol[0:8, 0:1],
                                         gh_all[:, :], start=True, stop=True)
                        rows = gh[0:1, ai * 16:(ai + 1) * 16, :]
                        nc.vector.tensor_copy(rows, g2[0:1, :])

                    for ci in range(NCH + 1):
                        if ci < NCH:
                            stage1(0, ci)
                        if ci >= 1:
                            stage2(0, ci - 1)
                    for ci in range(NCH + 1):
                        if ci < NCH:
                            stage1(1, ci)
                        if ci >= 1:
                            stage2(1, ci - 1)
                        if ci == 1:
                            extract(0)
                    extract(1)

                # ---------------- Phase C: all-reduce + chi2 ----------------
                with tc.tile_pool(name=f"pc{rep}", bufs=1) as pc:
                    # gh[0, a*16+jq, jv]: sim a=0 (H_w | H_wf), exp a=1 (cnt | H_f)
                    # partA[k=8q+r]: sim = H_w - H_wf ; exp = cnt - H_f
                    # partB[k]     : sim = H_wf      ; exp = H_f
                    paT = pc.tile([1, 2, BINS], F32, name="paT")
                    pbT = pc.tile([1, 2, BINS], F32, name="pbT")
                    for a in range(2):
                        nc.vector.tensor_tensor(
                            paT[0:1, a, :], gh[0:1, a * 16:(a + 1) * 16, 0:8],
                            gh[0:1, a * 16:(a + 1) * 16, 8:16], AL.subtract)
                        nc.vector.tensor_copy(
                            pbT[0:1, a, :], gh[0:1, a * 16:(a + 1) * 16, 8:16])
                    # hist[b] = partA[b] (b>=1) + partB[b-1] (b-1<=125); ends zero
                    hist = pc.tile([1, 2, BINS], F32, name="hist")
                    nc.vector.memset(hist[:], 0.0)
                    for a in range(2):
                        nc.vector.tensor_copy(
                            hist[0:1, a, 1:127], paT[0:1, a, 1:127])
                        nc.vector.tensor_tensor(
                            hist[0:1, a, 1:127], hist[0:1, a, 1:127],
                            pbT[0:1, a, 0:126], AL.add)
                    # normalize each by its sum, then chi2 / delta^2
                    ssum = pc.tile([1, 2], F32, name="ssum")
                    for a in range(2):
                        nc.vector.tensor_reduce(
                            ssum[0:1, a:a + 1], hist[0:1, a, :],
                            mybir.AxisListType.X, AL.add)
                        nc.vector.reciprocal(ssum[0:1, a:a + 1], ssum[0:1, a:a + 1])
                        nc.vector.tensor_scalar(
                            hist[0:1, a, :], hist[0:1, a, :],
                            ssum[0:1, a:a + 1], None, AL.mult)
                    dif = pc.tile([1, BINS], F32, name="dif")
                    nc.vector.tensor_tensor(
                        dif[:], hist[0:1, 0, :], hist[0:1, 1, :], AL.subtract)
                    nc.vector.tensor_tensor(dif[:], dif[:], dif[:], AL.mult)
                    chi = pc.tile([1, 1], F32, name="chi")
                    nc.vector.tensor_reduce(
                        chi[:], dif[:], mybir.AxisListType.X, AL.add)
                    idel = pc.tile([1, 1], F32, name="idel")
                    nc.vector.reciprocal(idel[:], dsc[:])
                    nc.vector.tensor_tensor(idel[:], idel[:], idel[:], AL.mult)
                    nc.vector.tensor_tensor(chi[:], chi[:], idel[:], AL.mult)
                    nc.gpsimd.dma_start(out_ext[:], chi[:])

    _split_sync_waits(nc, __import__("concourse.mybir", fromlist=["x"]),
                      strip_same_engine=strip_waits)
    return nc


_CACHE = {}


def _get_nc(repeat):
    if repeat not in _CACHE:
        _CACHE[repeat] = build(repeat=repeat)
    return _CACHE[repeat]


def kernel(**inputs):
    sim = np.ascontiguousarray(inputs["sim_observable"], dtype=np.float32)
    exp = np.ascontiguousarray(inputs["exp_observable"], dtype=np.float32)
    w = np.ascontiguousarray(inputs["weights"], dtype=np.float32)
    assert sim.shape == (N,) and exp.shape == (N,) and w.shape == (N,)

    from concourse.bass_utils import run_bass_kernel_spmd

    repeat = int(os.environ.get("BASS_HIST_REPEAT", "1"))
    nc = _get_nc(repeat)
    sim_s = sim.reshape(NCORES, P, FTOT)
    exp_s = exp.reshape(NCORES, P, FTOT)
    w_s = w.reshape(NCORES, P, FTOT)
    in_maps = [
        {"sim": sim_s[c], "exp": exp_s[c], "w": w_s[c]} for c in range(NCORES)
    ]
    res = run_bass_kernel_spmd(nc, in_maps, list(range(NCORES)))
    val = res.results[0]["out"][0, 0]
    return np.asarray(val, dtype=np.float32).reshape(())


# revision 34
# speedup vs baseline: 1.0280x; 1.0280x over previous
"""Trainium2 Bass kernel for nn_BinnedLoss (tent-weighted 128-bin chi2 loss).

O(N) factorized histogram: bin k = 8*q + r (q in [0,16), r in [0,8)).
Per 8-column block, the DVE builds a q-one-hot [128,16*8] (stationary) and
a value matrix [128,16*8] = [delta_r*a | delta_r*b] (moving); one PE matmul
per block accumulates all (q, r, stream) sums into a [128,128] PSUM tile
whose 8 diagonal 16x16 sub-blocks hold the histogram. Tent weights: a
sample with u=(x-mn)/step contributes w*(1-frac) to bin kf and w*frac to
bin kf+1 (tent continuity makes exact-floor tie behavior irrelevant, so
floor is computed as round(u-0.5) with fp32 magic-number rounding).

Work is spread across all five engines: Act does the scale/bias chains and
all bf16/int casts (incl. an Act-only floor(kf/8) chain for q), GPSIMD
(Pool) computes frac and part of the b-stream mults plus the cross-lane
max-reduce half of the min/max scan, PE contracts the one-hot blocks and
extracts diagonals via gather-matmuls, and per-array AllGathers (cheaper
than AllReduce) combine the per-core histograms, with the sim-array
collective overlapped under the exp-array compute. The chi2 tail mirrors
the reference math (normalize by sum, divide by delta, sum of squares).

kernel(**inputs) -> np.float32 scalar (shape ()).
"""
import os
import sys

sys.path.insert(0, "/opt/trn_rl_repo")
import numpy as np

N = 16777216
NCORES = 8
BINS = 128
P = 128
NSH = N // NCORES            # samples per core
FTOT = NSH // P              # 16384 free columns per core per array
FC = 1024                    # columns per chunk
NBLK = FC // 8               # matmul blocks per chunk
NCH = FTOT // FC             # chunks per array
MAGIC2 = 12582912.0          # 1.5*2^23: round-to-int magic for [0,128) range
QBIAS = -1572864.0           # -(12582912 >> 3)


def _patches(mybir, tile):
    from concourse.vector_clock import ScopedClock

    def _patched(self, tick_clock, wait_clock):
        drain_inst = self.nc.sync.drain()
        wait_clock.add_sem_waits(
            drain_inst.ins, ScopedClock({None: tick_clock.global_clock})
        )
        si = drain_inst.ins.sync_info
        if si is not None and si.on_wait and len(si.on_wait) > 1:
            waits = list(si.on_wait)
            drain_inst.ins.sync_info = mybir.SyncInfo(
                on_wait=[waits[0]], on_update=list(si.on_update)
            )
            for w in waits[1:]:
                nop = self.nc.sync.nop()
                nop.ins.sync_info = mybir.SyncInfo(on_wait=[w], on_update=[])
        self.nc.all_engine_barrier()
        assert self.sems is not None
        popped = self.nc._tile_sem_poison_stack.pop()
        assert popped is self._sem_poison
        self.nc.clear_and_free_semaphores(list(self.sems.allocated().values()))
        self.nc.all_engine_barrier()

    tile.TileContext._drain_and_barrier = _patched


def _split_sync_waits(nc, mybir, strip_same_engine=True):
    """Drop same-engine waits; hoist extra sem-waits onto same-engine NOPs
    (walrus allows <=1 sem-wait per instruction)."""
    eng_sem = {}
    counter = [0]
    for f in nc.m.functions:
        for bb in f.blocks:
            out = []
            dirty = False
            for inst in bb.instructions:
                si = inst.sync_info
                pref = eng_sem.get(inst.engine) if strip_same_engine else None
                if si is not None and si.on_wait and pref is not None:
                    kept = [
                        w for w in si.on_wait
                        if not (w.ant_name or "").startswith(pref + "_")
                    ]
                    if len(kept) != len(si.on_wait):
                        inst.sync_info = mybir.SyncInfo(
                            on_wait=kept, on_update=list(si.on_update))
                        si = inst.sync_info
                        dirty = True
                if si is not None and si.on_wait and len(si.on_wait) > 1:
                    waits = list(si.on_wait)
                    for w in waits[:-1]:
                        counter[0] += 1
                        nop = mybir.InstNoOp(
                            name=f"WSPLIT-{counter[0]}", ins=[], outs=[]
                        )
                        nop.engine = inst.engine
                        nop.sync_info = mybir.SyncInfo(on_wait=[w], on_update=[])
                        nc.register_instruction(nop, overwrite=True)
                        out.append(nop)
                    inst.sync_info = mybir.SyncInfo(
                        on_wait=[waits[-1]], on_update=list(si.on_update)
                    )
                    dirty = True
                out.append(inst)
            if dirty:
                bb.instructions = out


def build(repeat=1, strip_waits=True, CHUNK_A=4096, NPS=5, NPE=3):
    import concourse.bass as bass
    import concourse.mybir as mybir
    from concourse import tile

    _patches(mybir, tile)
    DT = mybir.dt
    AL = mybir.AluOpType
    ACT = mybir.ActivationFunctionType
    F32 = DT.float32
    BF16 = DT.bfloat16
    I32 = DT.int32
    core_ids = list(range(NCORES))

    nc = bass.Bass()
    sim_ext = nc.declare_dram_parameter("sim", [P, FTOT], F32, isOutput=False)
    exp_ext = nc.declare_dram_parameter("exp", [P, FTOT], F32, isOutput=False)
    w_ext = nc.declare_dram_parameter("w", [P, FTOT], F32, isOutput=False)
    out_ext = nc.declare_dram_parameter("out", [1, 1], F32, isOutput=True)

    with tile.TileContext(nc) as tc:
        with (
            tc.tile_pool(name="const", bufs=1) as cpool,
            tc.tile_pool(name="dram", bufs=1, space="DRAM") as dram,
            tc.tile_pool(name="psum", bufs=1, space="PSUM") as psum,
        ):
            cc_a_in = dram.tile([1, 2], F32, name="cc_a_in")
            cc_a_out = dram.tile([1, 16], F32, name="cc_a_out")
            cc_h_in = [dram.tile([1, 256], F32, name=f"cc_h_in{a}")
                       for a in range(2)]
            cc_h_out = [dram.tile([1, 2048], F32, name=f"cc_h_out{a}")
                        for a in range(2)]

            ones1 = cpool.tile([1, P], F32, name="ones1")
            nc.vector.memset(ones1[:], 1.0)
            # gather masks pdc[p, dc, jq] = (p == jq*8+dc)
            pidx = cpool.tile([P, 1], I32, name="pidx")
            nc.gpsimd.iota(pidx[:], [[1, 1]], channel_multiplier=1)
            pfl = cpool.tile([P, 1], F32, name="pfl")
            nc.vector.tensor_copy(pfl[:], pidx[:])
            pdc = cpool.tile([P, 8, 16], F32, name="pdc")
            for dc in range(8):
                for jq in range(16):
                    nc.vector.tensor_scalar(
                        pdc[:, dc, jq:jq + 1], pfl[:], float(jq * 8 + dc),
                        None, AL.is_equal)

            # scalars: sc = [inv, bias0]; dsc = delta
            sc = cpool.tile([1, 2], F32, name="sc")
            dsc = cpool.tile([1, 1], F32, name="dsc")
            bc = cpool.tile([P, 2], F32, name="bc")
            bcps = psum.tile([P, 2], F32, name="bcps")
            psA = psum.tile([P, P], F32, name="psA")
            psB = psum.tile([P, P], F32, name="psB")
            g0 = psum.tile([16, P], F32, name="g0")
            g1 = psum.tile([16, P], F32, name="g1")
            g2 = psum.tile([1, 256], F32, name="g2")

            cm2 = cpool.tile([P, 1], F32, name="cm2")
            nc.vector.memset(cm2[:], MAGIC2)
            cm2n = cpool.tile([P, 1], F32, name="cm2n")
            nc.vector.memset(cm2n[:], -MAGIC2)
            cq1 = cpool.tile([P, 1], F32, name="cq1")
            nc.vector.memset(cq1[:], -0.4375)
            cq2 = cpool.tile([P, 1], F32, name="cq2")
            nc.vector.memset(cq2[:], 0.125)
            gh = cpool.tile([1, 32, 16], F32, name="gh")
            onescol = cpool.tile([P, 1], F32, name="onescol")
            nc.vector.memset(onescol[:], 1.0)
            accs = [cpool.tile([16, 16], F32, name=f"acc{a}") for a in range(2)]

            for rep in range(repeat):
                # ---------------- Phase A: global min/max ----------------
                with tc.tile_pool(name=f"pa{rep}", bufs=2) as pa:
                    CW = CHUNK_A
                    NIT = FTOT // CW
                    rmin = pa.tile([P, 1], F32, name="rmin", bufs=1)
                    rmax = pa.tile([P, 1], F32, name="rmax", bufs=1)
                    # per-iter global maxes of sim (Pool XYZWC) land here
                    gmxs = pa.tile([1, NIT], F32, name="gmxs", bufs=1)
                    nc.vector.memset(rmin[:], 1.0e30)
                    nc.vector.memset(rmax[:], -1.0e30)
                    for it, cv in enumerate(range(0, FTOT, CW)):
                        chs = pa.tile([P, CW], F32, name="chs", tag="chs")
                        che = pa.tile([P, CW], F32, name="che", tag="che")
                        tmin = pa.tile([P, 1], F32, name="tmin", bufs=1)
                        tmin2 = pa.tile([P, 1], F32, name="tmin2", bufs=1)
                        tmax = pa.tile([P, 1], F32, name="tmax", bufs=1)
                        nc.sync.dma_start(chs[:], sim_ext[:, bass.ds(cv, CW)])
                        nc.sync.dma_start(che[:], exp_ext[:, bass.ds(cv, CW)])
                        # split engines: Pool takes sim's max (cross-lane
                        # reduce supports max); DVE takes both mins + exp max
                        nc.gpsimd.tensor_reduce(
                            gmxs[0:1, it:it + 1], chs[:],
                            mybir.AxisListType.XYZWC, AL.max)
                        nc.vector.tensor_reduce(
                            tmin[:], chs[:], mybir.AxisListType.X, AL.min)
                        nc.vector.tensor_reduce(
                            tmin2[:], che[:], mybir.AxisListType.X, AL.min)
                        nc.vector.tensor_reduce(
                            tmax[:], che[:], mybir.AxisListType.X, AL.max)
                        nc.vector.tensor_tensor(rmin[:], rmin[:], tmin[:], AL.min)
                        nc.vector.tensor_tensor(rmin[:], rmin[:], tmin2[:], AL.min)
                        nc.vector.tensor_tensor(rmax[:], rmax[:], tmax[:], AL.max)
                    # pm = [exp rmax (P) | sim gmxs (NIT) | rmin (P)]
                    pm = pa.tile([1, 2 * P + NIT], F32, name="pm", bufs=1)
                    nc.gpsimd.dma_start(pm[0:1, 0:P], rmax[:, 0:1])
                    nc.vector.tensor_copy(pm[0:1, P:P + NIT], gmxs[:])
                    nc.gpsimd.dma_start(pm[0:1, P + NIT:2 * P + NIT], rmin[:, 0:1])
                    pk = pa.tile([1, 2], F32, name="pk", bufs=1)
                    nc.vector.tensor_reduce(
                        pk[0:1, 0:1], pm[0:1, 0:P + NIT],
                        mybir.AxisListType.X, AL.max)
                    nc.vector.tensor_reduce(
                        pk[0:1, 1:2], pm[0:1, P + NIT:2 * P + NIT],
                        mybir.AxisListType.X, AL.min)
                    nc.vector.tensor_scalar_mul(pk[0:1, 1:2], pk[0:1, 1:2], -1.0)
                    nc.gpsimd.dma_start(cc_a_in[:], pk[:])
                    nc.gpsimd.collective_compute(
                        "AllGather", AL.bypass, replica_groups=[core_ids],
                        ins=[cc_a_in.opt()], outs=[cc_a_out.opt()],
                    )
                    ga_all = pa.tile([1, 16], F32, name="ga_all", bufs=1)
                    nc.gpsimd.dma_start(ga_all[:], cc_a_out[:])
                    ga = pa.tile([1, 2], F32, name="ga", bufs=1)
                    nc.vector.tensor_reduce(
                        ga[0:1, 0:1], ga_all[0:1, bass.ds(0, 8, 2)],
                        mybir.AxisListType.X, AL.max)
                    nc.vector.tensor_reduce(
                        ga[0:1, 1:2], ga_all[0:1, bass.ds(1, 8, 2)],
                        mybir.AxisListType.X, AL.max)
                    # ga = [mx, -mn]; d = mx - mn
                    mnt = pa.tile([1, 1], F32, name="mnt", bufs=1)
                    d_t = pa.tile([1, 1], F32, name="d_t", bufs=1)
                    st_t = pa.tile([1, 1], F32, name="st_t", bufs=1)
                    nc.vector.tensor_scalar_mul(mnt[:], ga[0:1, 1:2], -1.0)
                    nc.vector.tensor_tensor(d_t[:], ga[0:1, 0:1], mnt[:], AL.subtract)
                    nc.vector.tensor_scalar_mul(
                        st_t[:], d_t[:], float(np.float32(1.0) / np.float32(127.0)))
                    nc.vector.reciprocal(sc[0:1, 0:1], st_t[:])      # inv
                    nc.vector.scalar_tensor_tensor(
                        sc[0:1, 1:2], mnt[:], -1.0, sc[0:1, 0:1],
                        AL.mult, AL.mult)                            # -mn*inv
                    nc.vector.tensor_scalar_add(sc[0:1, 1:2], sc[0:1, 1:2], -0.5)
                    nc.vector.tensor_scalar_mul(dsc[:], d_t[:], 0.0078125)
                    nc.tensor.matmul(bcps[:], ones1[:], sc[0:1, :],
                                     start=True, stop=True)
                    nc.vector.tensor_copy(bc[:], bcps[:])

                # ---------------- Phase B ----------------
                ARR = ((sim_ext, True, psA), (exp_ext, False, psB))
                with (
                    tc.tile_pool(name=f"st{rep}", bufs=2) as st,
                    tc.tile_pool(name=f"pw{rep}", bufs=2) as pw,
                    tc.tile_pool(name=f"oh{rep}", bufs=2) as oh,
                    tc.tile_pool(name=f"px{rep}", bufs=1) as px,
                ):
                    # two-stage software pipeline per array: stage1(ci) feeds
                    # the engines (DMA, Act scale/bias chain, casts); stage2
                    # (ci-1) consumes (DVE one-hots + a-stream, Pool b-stream
                    # mults, PE matmuls). The sim histogram's extraction and
                    # AllGather are emitted two chunks into the exp pipeline
                    # so the collective overlaps exp compute.
                    casts = {}

                    def stage1(ai, ci):
                        arr, weighted, ps = ARR[ai]
                        x = st.tile([P, FC], F32, name="x", tag="x")
                        nc.sync.dma_start(x[:], arr[:, bass.ds(ci * FC, FC)])
                        wt = None
                        if weighted:
                            wt = st.tile([P, FC], F32, name="wt", tag="wt")
                            nc.sync.dma_start(
                                wt[:], w_ext[:, bass.ds(ci * FC, FC)])
                        u = pw.tile([P, FC], F32, name="u", tag="u")
                        t = pw.tile([P, FC], F32, name="t", tag="t", bufs=1)
                        kf = pw.tile([P, FC], F32, name="kf", tag="kf", bufs=1)
                        fr = pw.tile([P, FC], F32, name="fr", tag="fr", bufs=1)
                        ki = pw.tile([P, FC], I32, name="ki", tag="ki", bufs=1)
                        qi_f = pw.tile([P, FC], F32, name="qi_f", tag="qi", bufs=1)
                        qi_f2 = pw.tile([P, FC], F32, name="qi_f2", tag="qi2", bufs=1)
                        ri = pw.tile([P, FC], I32, name="ri", tag="ri", bufs=1)
                        qb = pw.tile([P, NBLK, 8], BF16, name="qb", tag="qb")
                        rb = pw.tile([P, NBLK, 8], BF16, name="rb", tag="rb")
                        nc.scalar.activation(
                            u[:], x[:], ACT.Identity,
                            bias=bc[:, 1:2], scale=bc[:, 0:1])
                        nc.scalar.activation(t[:], u[:], ACT.Identity,
                                             bias=cm2[:, 0:1])
                        nc.scalar.activation(kf[:], t[:], ACT.Identity,
                                             bias=cm2n[:, 0:1])
                        nc.scalar.activation(ki[:], t[:], ACT.Copy)
                        nc.vector.tensor_scalar(
                            ri[:], ki[:], 7, None, AL.bitwise_and)
                        # q via Act-only: a1 = kf/8 - 0.4375 (exact),
                        # a2 = round(a1) + M2 (magic), qb = a2 - M2
                        nc.scalar.activation(qi_f[:], kf[:], ACT.Identity,
                                             bias=cq1[:, 0:1], scale=cq2[:, 0:1])
                        nc.scalar.activation(qi_f2[:], qi_f[:], ACT.Identity,
                                             bias=cm2[:, 0:1])
                        nc.scalar.activation(qb[:], qi_f2[:], ACT.Copy,
                                             bias=-MAGIC2)
                        nc.scalar.activation(rb[:], ri[:], ACT.Copy)
                        nc.gpsimd.tensor_tensor(fr[:], u[:], kf[:], AL.subtract)
                        fb = pw.tile([P, NBLK, 8], BF16, name="fb", tag="fb")
                        nc.scalar.activation(fb[:], fr[:], ACT.Copy, bias=0.5)
                        va = None
                        if weighted:
                            va = pw.tile([P, NBLK, 8], BF16, name="va", tag="va")
                            nc.scalar.activation(va[:], wt[:], ACT.Copy)
                        casts[(ai, ci)] = (qb, rb, fb, va)

                    def stage2(ai, ci):
                        arr, weighted, ps = ARR[ai]
                        qb, rb, fb, va = casts.pop((ai, ci))
                        QOH = oh.tile([P, NBLK, 16, 8], BF16, name="QOH", tag="QOH")
                        V = oh.tile([P, NBLK, 16, 8], BF16, name="V", tag="V")
                        for jq in range(16):
                            nc.vector.tensor_scalar(
                                QOH[:, :, jq, :], qb[:], float(jq),
                                None, AL.is_equal)
                        if weighted:
                            ROH = oh.tile([P, NBLK, 8], BF16, name="ROH",
                                          tag="ROH", bufs=1)
                            for jr in range(8):
                                nc.vector.tensor_scalar(
                                    ROH[:], rb[:], float(jr), None, AL.is_equal)
                                nc.vector.tensor_tensor(
                                    V[:, :, jr, :], ROH[:], va[:], AL.mult)
                        else:
                            for jr in range(8):
                                nc.vector.tensor_scalar(
                                    V[:, :, jr, :], rb[:], float(jr),
                                    None, AL.is_equal)
                        npool = NPS if weighted else NPE
                        for jr in range(8):
                            eng = nc.gpsimd if jr < npool else nc.vector
                            eng.tensor_tensor(
                                V[:, :, 8 + jr, :], V[:, :, jr, :],
                                fb[:], AL.mult)
                        for b in range(NBLK):
                            nc.tensor.matmul(
                                ps[:, :], QOH[:, b, :, :], V[:, b, :, :],
                                start=(ci == 0 and b == 0),
                                stop=(ci == NCH - 1 and b == NBLK - 1))

                    def extract(ai):
                        # diagonal blocks acc[jq,jv] = sum_dc ps[jq*8+dc, jv*8+dc],
                        # then AllGather + cross-core sum into gh rows
                        ps = ARR[ai][2]
                        Sf = px.tile([P, P], F32, name="Sf", tag="Sf")
                        nc.vector.tensor_copy(Sf[:], ps[:, :])
                        acc = accs[ai]
                        for dc in range(8):
                            g = (g0, g1)[dc % 2]
                            nc.tensor.matmul(g[:, :], pdc[:, dc, :], Sf[:, :],
                                             start=True, stop=True)
                            if dc == 0:
                                nc.vector.tensor_copy(acc[:], g[:, bass.ds(dc, 16, 8)])
                            else:
                                nc.vector.tensor_tensor(
                                    acc[:], acc[:], g[:, bass.ds(dc, 16, 8)], AL.add)
                        nc.gpsimd.dma_start(cc_h_in[ai][:], acc[:, :])
                        nc.gpsimd.collective_compute(
                            "AllGather", AL.bypass, replica_groups=[core_ids],
                            ins=[cc_h_in[ai].opt()], outs=[cc_h_out[ai].opt()],
                        )
                        gh_all = px.tile([8, 256], F32, name="gh_all",
                                         tag="gh_all")
                        nc.gpsimd.dma_start(gh_all[:], cc_h_out[ai][:])
                        nc.tensor.matmul(g2[:, :], onescol[0:8, 0:1],
                                         gh_all[:, :], start=True, stop=True)
                        rows = gh[0:1, ai * 16:(ai + 1) * 16, :]
                        nc.vector.tensor_copy(rows, g2[0:1, :])

                    for ci in range(NCH):
                        stage1(0, ci)
                        if ci >= 1:
                            stage2(0, ci - 1)
                    stage1(1, 0)
                    stage2(0, NCH - 1)
                    stage1(1, 1)
                    stage2(1, 0)
                    extract(0)
                    for ci in range(2, NCH + 1):
                        if ci < NCH:
                            stage1(1, ci)
                        stage2(1, ci - 1)
                    extract(1)

                # ---------------- Phase C: all-reduce + chi2 ----------------
                with tc.tile_pool(name=f"pc{rep}", bufs=1) as pc:
                    # gh[0, a*16+jq, jv]: sim a=0 (H_w | H_wf), exp a=1 (cnt | H_f)
                    # partA[k=8q+r]: sim = H_w - H_wf ; exp = cnt - H_f
                    # partB[k]     : sim = H_wf      ; exp = H_f
                    paT = pc.tile([1, 2, BINS], F32, name="paT")
                    pbT = pc.tile([1, 2, BINS], F32, name="pbT")
                    for a in range(2):
                        nc.vector.tensor_tensor(
                            paT[0:1, a, :], gh[0:1, a * 16:(a + 1) * 16, 0:8],
                            gh[0:1, a * 16:(a + 1) * 16, 8:16], AL.subtract)
                        nc.vector.tensor_copy(
                            pbT[0:1, a, :], gh[0:1, a * 16:(a + 1) * 16, 8:16])
                    # hist[b] = partA[b] (b>=1) + partB[b-1] (b-1<=125); ends zero
                    hist = pc.tile([1, 2, BINS], F32, name="hist")
                    nc.vector.memset(hist[:], 0.0)
                    for a in range(2):
                        nc.vector.tensor_copy(
                            hist[0:1, a, 1:127], paT[0:1, a, 1:127])
                        nc.vector.tensor_tensor(
                            hist[0:1, a, 1:127], hist[0:1, a, 1:127],
                            pbT[0:1, a, 0:126], AL.add)
                    # normalize each by its sum, then chi2 / delta^2
                    ssum = pc.tile([1, 2], F32, name="ssum")
                    for a in range(2):
                        nc.vector.tensor_reduce(
                            ssum[0:1, a:a + 1], hist[0:1, a, :],
                            mybir.AxisListType.X, AL.add)
                        nc.vector.reciprocal(ssum[0:1, a:a + 1], ssum[0:1, a:a + 1])
                        nc.vector.tensor_scalar(
                            hist[0:1, a, :], hist[0:1, a, :],
                            ssum[0:1, a:a + 1], None, AL.mult)
                    dif = pc.tile([1, BINS], F32, name="dif")
                    nc.vector.tensor_tensor(
                        dif[:], hist[0:1, 0, :], hist[0:1, 1, :], AL.subtract)
                    nc.vector.tensor_tensor(dif[:], dif[:], dif[:], AL.mult)
                    chi = pc.tile([1, 1], F32, name="chi")
                    nc.vector.tensor_reduce(
                        chi[:], dif[:], mybir.AxisListType.X, AL.add)
                    idel = pc.tile([1, 1], F32, name="idel")
                    nc.vector.reciprocal(idel[:], dsc[:])
                    nc.vector.tensor_tensor(idel[:], idel[:], idel[:], AL.mult)
                    nc.vector.tensor_tensor(chi[:], chi[:], idel[:], AL.mult)
                    nc.gpsimd.dma_start(out_ext[:], chi[:])

    _split_sync_waits(nc, __import__("concourse.mybir", fromlist=["x"]),
                      strip_same_engine=strip_waits)
    return nc


_CACHE = {}


def _get_nc(repeat):
    if repeat not in _CACHE:
        _CACHE[repeat] = build(repeat=repeat)
    return _CACHE[repeat]


def kernel(**inputs):
    sim = np.ascontiguousarray(inputs["sim_observable"], dtype=np.float32)
    exp = np.ascontiguousarray(inputs["exp_observable"], dtype=np.float32)
    w = np.ascontiguousarray(inputs["weights"], dtype=np.float32)
    assert sim.shape == (N,) and exp.shape == (N,) and w.shape == (N,)

    from concourse.bass_utils import run_bass_kernel_spmd

    repeat = int(os.environ.get("BASS_HIST_REPEAT", "1"))
    nc = _get_nc(repeat)
    sim_s = sim.reshape(NCORES, P, FTOT)
    exp_s = exp.reshape(NCORES, P, FTOT)
    w_s = w.reshape(NCORES, P, FTOT)
    in_maps = [
        {"sim": sim_s[c], "exp": exp_s[c], "w": w_s[c]} for c in range(NCORES)
    ]
    res = run_bass_kernel_spmd(nc, in_maps, list(range(NCORES)))
    val = res.results[0]["out"][0, 0]
    return np.asarray(val, dtype=np.float32).reshape(())
